# revision 1
# baseline (speedup 1.0000x reference)
"""Trainium2 Bass kernel for nn_AggregateConcatenate.

out[b] = concat([masked {mean,max,min,std} of tanh-MLP_agg(x_b) over the valid
prefix, tanh-MLP_adj(x_b)], axis=1)

Sharding: data-parallel over batch B across 8 NeuronCores (4 bags/core).
Bags are clustered by length into 4 slots (8 similar-length bags share a
slot across cores); slots are processed longest-first so the kernel tail is
adj-only matmul work under which the ragged finalization drains.

Per-core dataflow (matmuls in float32r: full PE rate at N>=256):
  - mm1 feature-major: h^T[h, tok] = W1T.T @ x^T (x pre-transposed on host),
    clipped per-slot to ceil(valid/128) tokens (width >= 256 keeps full rate).
  - ELU via the exact identity elu(z) = min(exp(z) - 1, relu(z)):
    Exp and Relu on ScalarE (bias b1 fused), one scalar_tensor_tensor on
    VectorE. Keeps VectorE comfortably faster than the PE mm1 stream.
  - mm2 token-major: q[tok, a] = hT.T @ W2T -> adjacent output rows DMA out
    contiguously; the ragged reductions see tokens on partitions.
  - masked sum / sum-of-squares via bf16 PE matmuls with per-(bag, tile) mask
    columns stationary, col-tiled so the 4 token tiles of a chunk run in
    different PE column groups concurrently; folded at the end with one
    select-matmul. max/min via bf16 candidates folded into per-bag bf16
    accumulators on VectorE, transposed with regular pipelined bf16 matmuls
    against an identity and reduced on VectorE.
  - startup: the first bag runs agg-only phases for chunks 0-1 before their
    adj phases, so the early PE demand (w1_agg + x + w2_agg = 6MB over 40us)
    stays within what the two HWDGE queues deliver; weights stream in
    first-use-order chunks while the PE warms the HAM clock gate on junk
    matmuls.
"""

import numpy as np

import concourse.bass as bass
import concourse.tile as tile
from concourse import mybir
from concourse.bass_utils import run_bass_kernel_spmd
from concourse.tile import ScopedClock

B, T, E, H, A = 32, 2048, 512, 1024, 512
NCORES = 8
BPC = B // NCORES          # bags per core (= slots)
NCH = T // 512             # 512-token chunks per bag
NTT = 4                    # 128-token tiles per chunk
KE = E // 128              # k-tiles for mm1
KH = H // 128              # k-tiles for mm2
NHT = H // 128             # h-tiles (partition tiles of H)
BIG = 30.0                 # ragged padding offset; |tanh| < 1 << BIG

F32 = mybir.dt.float32
F32R = mybir.dt.float32r
BF16 = mybir.dt.bfloat16
AF = mybir.ActivationFunctionType
OP = mybir.AluOpType


class _SplitDrainTileContext(tile.TileContext):
    """TileContext whose exit drain splits sem waits across sync NOPs."""

    WAIT_LIMIT = 1

    def _drain_and_barrier(self, tick_clock, wait_clock):
        drain_inst = self.nc.sync.drain()
        wait_clock.add_sem_waits(
            drain_inst.ins, ScopedClock({None: tick_clock.global_clock})
        )
        si = drain_inst.ins.sync_info
        if si is not None and len(si.on_wait) > self.WAIT_LIMIT:
            waits = list(si.on_wait)
            drain_inst.ins.sync_info = mybir.SyncInfo(
                on_wait=waits[: self.WAIT_LIMIT], on_update=list(si.on_update)
            )
            for i in range(self.WAIT_LIMIT, len(waits), self.WAIT_LIMIT):
                nop = self.nc.sync.nop()
                nop.ins.sync_info = mybir.SyncInfo(
                    on_wait=waits[i : i + self.WAIT_LIMIT], on_update=[]
                )
        self.nc.all_engine_barrier()
        assert self.sems is not None
        popped = self.nc._tile_sem_poison_stack.pop()
        assert popped is self._sem_poison
        self.nc.clear_and_free_semaphores(list(self.sems.allocated().values()))
        self.nc.all_engine_barrier()


def _split_waits(nc, limit: int = 1):
    """Walrus codegen accepts at most one sync wait per TPB instruction.

    Hoist excess waits from any instruction onto injected same-engine NOPs
    placed immediately before it (same-engine program order is preserved, so
    waiting earlier is equivalent).
    """
    uid = [0]
    for f in nc.m.functions:
        for bb in f.blocks:
            new_insts = []
            for ins in bb.instructions:
                si = ins.sync_info
                if si is not None and len(si.on_wait) > limit:
                    waits = list(si.on_wait)
                    keep = waits[:limit]
                    rest = waits[limit:]
                    for j in range(0, len(rest), limit):
                        uid[0] += 1
                        nop = mybir.InstNoOp(
                            name=f"waitnop-{uid[0]}",
                            engine=ins.engine,
                            ins=[],
                            outs=[],
                        )
                        nop.sync_info = mybir.SyncInfo(
                            on_wait=rest[j : j + limit], on_update=[]
                        )
                        new_insts.append(nop)
                    ins.sync_info = mybir.SyncInfo(
                        on_wait=keep, on_update=list(si.on_update)
                    )
                new_insts.append(ins)
            if len(new_insts) != len(bb.instructions):
                bb.instructions = new_insts
    return nc


def _widths(tct: int) -> list:
    """mm1 chunk widths covering ceil(tct) 128-token tiles, each >= 256."""
    full, rem = divmod(tct, NTT)
    w = [512] * full
    if rem:
        w.append(max(256, rem * 128))
    return w


def _build_program(with_b2: bool, b1_zero: bool, tct: tuple, order: tuple):
    nc = bass.Bass()

    # all tensors arrive pre-tiled to the exact SBUF layout so every DMA is
    # a contiguous max-line-size copy
    xt = nc.declare_dram_parameter("xt", [BPC, NCH, 128, KE, 512], F32R, isOutput=False)
    w1t = {}
    w2t = {}
    b1 = {}
    b2 = {}
    for m in ("agg", "adj"):
        w1t[m] = nc.declare_dram_parameter(
            f"w1t_{m}", [128, NHT, KE, 128], F32R, isOutput=False
        )
        w2t[m] = nc.declare_dram_parameter(
            f"w2t_{m}", [128, KH, A], F32R, isOutput=False
        )
        if not b1_zero:
            b1[m] = nc.declare_dram_parameter(f"b1_{m}", [128, NHT], F32, isOutput=False)
        if with_b2:
            b2[m] = nc.declare_dram_parameter(f"b2_{m}", [1, A], F32R, isOutput=False)
    maskd = nc.declare_dram_parameter("mask", [128, BPC, 16], F32, isOutput=False)
    negmaskd = nc.declare_dram_parameter("negmask", [128, BPC, 16], F32, isOutput=False)
    negpadd = nc.declare_dram_parameter("negpad", [128, BPC, 16], F32, isOutput=False)
    slhsd = nc.declare_dram_parameter("slhs", [128, BPC, 16, BPC], BF16, isOutput=False)
    seld = nc.declare_dram_parameter("sel4", [128, BPC], F32R, isOutput=False)
    identd = nc.declare_dram_parameter("identb", [128, 128], BF16, isOutput=False)
    invnd = nc.declare_dram_parameter("inv_n", [BPC, 1], F32, isOutput=False)
    invnm1d = nc.declare_dram_parameter("inv_nm1", [BPC, 1], F32, isOutput=False)
    novernm1d = nc.declare_dram_parameter("n_over_nm1", [BPC, 1], F32, isOutput=False)
    out = nc.declare_dram_parameter("out", [BPC, 4 + T, A], F32, isOutput=True)

    widths = {s: _widths(tct[s]) for s in range(BPC)}
    # stat matmuls per col-group position j (tile index within chunk)
    pos_total = [0] * NTT
    for s in range(BPC):
        for j in range(NTT):
            pos_total[j] += max(0, -(-(tct[s] - j) // NTT)) if tct[s] > j else 0

    s0 = order[0]
    split_bag0 = len(widths[s0]) >= 2

    with _SplitDrainTileContext(nc) as tc:
        with (
            tc.tile_pool(name="consts", bufs=1) as consts,
            tc.tile_pool(name="accs", bufs=1) as accs,
            tc.tile_pool(name="xin", bufs=(3 if split_bag0 else 2)) as xin,
            tc.tile_pool(name="elu", bufs=2) as elu,
            tc.tile_pool(name="ht", bufs=1) as htp,
            tc.tile_pool(name="qp", bufs=1) as qp,
            tc.tile_pool(name="adjo", bufs=2) as adjo,
            tc.tile_pool(name="fin", bufs=1) as fin,
            tc.tile_pool(name="pb1", bufs=(2 if b1_zero else 4), space="PSUM") as pb1p,
            tc.tile_pool(name="pb2", bufs=2, space="PSUM") as pb2p,
            tc.tile_pool(name="pstat", bufs=1, space="PSUM") as pstat,
        ):
            # ---- DVE-side init first so the PE warmup can start ASAP ------
            junk = consts.tile([128, 512], F32, tag="junk", name="junk")
            nc.vector.memset(junk, 0.001)
            junk_r = junk.bitcast(F32R)
            warm_sb = consts.tile([1, 1], F32, tag="warm", name="warm")
            nc.vector.memset(warm_sb, 0.0)

            # first x chunk split by k-tiles for fastest time-to-first-matmul
            xb00 = xin.tile([128, KE, 512], F32R, tag="xb", name="xb00")
            nc.sync.dma_start(out=xb00[:, 0:1, :], in_=xt[s0, 0, :, 0:1, :])
            nc.sync.dma_start(out=xb00[:, 1:2, :], in_=xt[s0, 0, :, 1:2, :])
            nc.sync.dma_start(out=xb00[:, 2:4, :], in_=xt[s0, 0, :, 2:4, :])

            # ---- weights: first-use-order chunks across both HWDGE queues -
            w1sb = {}
            w2sb = {}
            b1sb = {}
            b2sb = {}
            for m in ("agg", "adj"):
                w1sb[m] = consts.tile(
                    [128, NHT, KE, 128], F32R, tag=f"w1_{m}", name=f"w1_{m}"
                )
                w2sb[m] = consts.tile([128, KH, A], F32R, tag=f"w2_{m}", name=f"w2_{m}")
            # agg weights on the ACT queue (scalar engine is free this early);
            # chunk sizes track the mm1/mm2 consumption rate
            # h0-h3 on ACT; h4-h7 ride the SP queue right after the x
            # prefetch so both queues feed mm1_agg in parallel
            for h0 in range(4):
                nc.scalar.dma_start(
                    out=w1sb["agg"][:, h0 : h0 + 1, :, :],
                    in_=w1t["agg"][:, h0 : h0 + 1, :, :],
                )
            for h0, h1 in ((4, 6), (6, 8)):
                nc.sync.dma_start(
                    out=w1sb["agg"][:, h0:h1, :, :], in_=w1t["agg"][:, h0:h1, :, :]
                )
            # the second x chunk + remaining weights behind the prefetch,
            # with w2_agg split across both queues so neither serializes
            xb01 = xin.tile([128, KE, 512], F32R, tag="xb", name="xb01")
            nc.sync.dma_start(out=xb01[:, 0:2, :], in_=xt[s0, 1, :, 0:2, :])
            nc.sync.dma_start(out=xb01[:, 2:4, :], in_=xt[s0, 1, :, 2:4, :])
            nc.scalar.dma_start(out=w2sb["agg"][:, 0:2, :], in_=w2t["agg"][:, 0:2, :])
            for k0, k1 in ((2, 4), (4, 6), (6, 8)):
                nc.sync.dma_start(
                    out=w2sb["agg"][:, k0:k1, :], in_=w2t["agg"][:, k0:k1, :]
                )
            for h0, h1 in ((0, 4), (4, 8)):
                nc.sync.dma_start(
                    out=w1sb["adj"][:, h0:h1, :, :], in_=w1t["adj"][:, h0:h1, :, :]
                )
            for k0, k1 in ((0, 4), (4, 8)):
                nc.sync.dma_start(
                    out=w2sb["adj"][:, k0:k1, :], in_=w2t["adj"][:, k0:k1, :]
                )

            # small consts on the SWDGE queue
            mask_sb = consts.tile([128, BPC, 16], F32, tag="mask", name="mask")
            nc.gpsimd.dma_start(out=mask_sb, in_=maskd[:, :, :])
            negmask_sb = consts.tile([128, BPC, 16], F32, tag="negmask", name="negmask")
            nc.gpsimd.dma_start(out=negmask_sb, in_=negmaskd[:, :, :])
            negpad_sb = consts.tile([128, BPC, 16], F32, tag="negpad", name="negpad")
            nc.gpsimd.dma_start(out=negpad_sb, in_=negpadd[:, :, :])
            slhs_sb = consts.tile([128, BPC, 16, BPC], BF16, tag="slhs", name="slhs")
            nc.gpsimd.dma_start(out=slhs_sb, in_=slhsd[:, :, :, :])
            sel_sb = consts.tile([128, BPC], F32R, tag="sel4", name="sel4")
            nc.gpsimd.dma_start(out=sel_sb, in_=seld[:, :])
            ident_b = consts.tile([128, 128], BF16, tag="identb", name="identb")
            nc.gpsimd.dma_start(out=ident_b, in_=identd[:, :])
            invn_sb = consts.tile([BPC, 1], F32, tag="invn", name="invn")
            nc.gpsimd.dma_start(out=invn_sb, in_=invnd[:, :])
            invnm1_sb = consts.tile([BPC, 1], F32, tag="invnm1", name="invnm1")
            nc.gpsimd.dma_start(out=invnm1_sb, in_=invnm1d[:, :])
            novernm1_sb = consts.tile([BPC, 1], F32, tag="novernm1", name="novernm1")
            nc.gpsimd.dma_start(out=novernm1_sb, in_=novernm1d[:, :])
            for m in ("agg", "adj"):
                if not b1_zero:
                    b1sb[m] = consts.tile([128, NHT], F32, tag=f"b1_{m}", name=f"b1_{m}")
                    nc.gpsimd.dma_start(out=b1sb[m], in_=b1[m][:, :])
                if with_b2:
                    b2sb[m] = consts.tile([1, A], F32R, tag=f"b2_{m}", name=f"b2_{m}")
                    nc.gpsimd.dma_start(out=b2sb[m], in_=b2[m][:, :])
            if with_b2:
                ones_col = consts.tile([1, 128], F32R, tag="ones", name="ones")
                nc.gpsimd.memset(ones_col, 1.0)

            # per-bag running accumulators for max(q) and max(-q), bf16 so
            # the final transposes run as full-rate bf16 matmuls
            acc_max = accs.tile([128, BPC, A], BF16, tag="acc_max", name="acc_max")
            nc.gpsimd.memset(acc_max, -2.0)
            acc_nmax = accs.tile([128, BPC, A], BF16, tag="acc_nmax", name="acc_nmax")
            nc.gpsimd.memset(acc_nmax, -2.0)

            # pre-warm the ACT exp table set during the DMA preamble
            nc.scalar.activation(warm_sb, warm_sb, AF.Exp)

            # warm the PE HAM clock gate while the first weights stream in
            # 14 junk matmuls: >3.4us of sustained PE activity trips the HAM
            # clock gate to full speed, and they productively burn the
            # window where the first weight chunks are still in flight
            pwarm = pb2p.tile([128, A], F32, tag="pb2", name="pwarm")
            for _ in range(14):
                nc.tensor.matmul(
                    pwarm, lhsT=junk_r[:, 0:128], rhs=junk_r, start=True, stop=True
                )

            # stats accumulators in PSUM: col-group j (tile-in-chunk) keeps
            # its partial sums at partitions 32j+bag; memset so the fold
            # copy never reads uninitialized PSUM on unused partitions
            psum_s = pstat.tile([128, A], F32, tag="psum_s", name="psum_s")
            nc.vector.memset(psum_s, 0.0)
            psum_q = pstat.tile([128, A], F32, tag="psum_q", name="psum_q")
            nc.vector.memset(psum_q, 0.0)

            # ---- emitters --------------------------------------------------
            pending_stats = []
            pos_cnt_s = [0] * NTT
            pos_cnt_q = [0] * NTT
            hgrp = 2 if b1_zero else 1

            def emit_stats(item):
                _, s, c, pq, pq2, ntl = item
                for psum, pqx, cnts in (
                    (psum_s, pq, pos_cnt_s), (psum_q, pq2, pos_cnt_q)
                ):
                    for tt in range(ntl):
                        tg = c * NTT + tt
                        lhs = slhs_sb[:, s, tg, :]
                        tp = (0, 32 * tt) if tt else None
                        nc.tensor.matmul(
                            psum[32 * tt : 32 * tt + BPC, :], lhsT=lhs,
                            rhs=pqx[:, tt, :],
                            start=(cnts[tt] == 0), stop=(cnts[tt] == pos_total[tt] - 1),
                            tile_position=tp, skip_group_check=True,
                        )
                        cnts[tt] += 1

            def emit_fin(s, last=False):
                # cross-partition max: transpose 128x128 blocks of the bf16
                # accumulators with regular pipelined matmuls against an
                # identity, reduce over the free axis on VectorE
                for acc, row, neg in ((acc_max, 1, False), (acc_nmax, 2, True)):
                    redt = fin.tile(
                        [128, NTT], BF16, tag="redt", name=f"redt_{s}_{row}", bufs=2
                    )
                    for ch in range(NTT):
                        # one PSUM-ring tile per transpose so consecutive
                        # transposes alternate banks instead of ping-ponging
                        # against the reduce on a single bank; the last fin
                        # also borrows the then-idle pb1 ring for 4-deep
                        if last and ch % 2:
                            pt = pb1p.tile([128, 128], F32, tag="pb1", name="pt_fin")
                        else:
                            pt = pb2p.tile([128, 128], F32, tag="pb2", name="pt_fin")
                        nc.tensor.matmul(
                            pt, lhsT=acc[:, s, ch * 128 : (ch + 1) * 128],
                            rhs=ident_b, start=True, stop=True,
                            skip_group_check=True,
                        )
                        nc.vector.tensor_reduce(
                            redt[:, ch : ch + 1], pt,
                            axis=mybir.AxisListType.X, op=OP.max,
                        )
                    prow = pb2p.tile([NTT, 128], F32, tag="pb2", name="prow_fin")
                    nc.tensor.matmul(
                        prow, lhsT=redt, rhs=ident_b, start=True, stop=True,
                        skip_group_check=True,
                    )
                    row_sb = fin.tile(
                        [NTT, 128], F32, tag="row", name=f"row_{s}_{row}", bufs=2
                    )
                    nc.scalar.mul(row_sb, prow, -1.0 if neg else 1.0)
                    nc.sync.dma_start(
                        out=out[s, row : row + 1, :].rearrange(
                            "o (c f) -> (o c) f", c=NTT
                        ),
                        in_=row_sb,
                    )

            def emit_mm1(s, c, m, xb, wm):
                htsb = htp.tile(
                    [128, KH, 512], F32R, tag=f"ht_{m}", name=f"ht_{m}",
                    bufs=(2 if (split_bag0 and m == "agg") else 1),
                )
                for hg in range(NHT // hgrp):
                    pb1 = pb1p.tile([128, hgrp, 512], F32, tag="pb1", name="pb1")
                    for j in range(hgrp):
                        ht = hg * hgrp + j
                        for kt in range(KE):
                            nc.tensor.matmul(
                                pb1[:, j, :wm],
                                lhsT=w1sb[m][:, ht, kt, :],
                                rhs=xb[:, kt, :wm],
                                start=(kt == 0),
                                stop=(kt == KE - 1),
                            )
                    bias = 0.0 if b1_zero else b1sb[m][:, hg : hg + 1]
                    pb1s = pb1[:, :, :wm]
                    e_sb = elu.tile([128, hgrp, 512], F32, tag="e", name="e")
                    nc.scalar.activation(e_sb[:, :, :wm], pb1s, AF.Exp, bias=bias)
                    r_sb = elu.tile([128, hgrp, 512], F32, tag="r", name="r")
                    nc.scalar.activation(r_sb[:, :, :wm], pb1s, AF.Relu, bias=bias)
                    nc.vector.scalar_tensor_tensor(
                        out=htsb[:, hg * hgrp : (hg + 1) * hgrp, :wm],
                        in0=e_sb[:, :, :wm], scalar=-1.0, in1=r_sb[:, :, :wm],
                        op0=OP.add, op1=OP.min,
                    )
                return htsb

            def emit_mm2_agg(s, c, htsb, ntl):
                q_sb = qp.tile([128, NTT, A], BF16, tag="q", name="q")
                q2_sb = qp.tile([128, NTT, A], BF16, tag="q2", name="q2")
                for tt in range(ntl):
                    pb2 = pb2p.tile([128, A], F32, tag="pb2", name="pb2")
                    for kt in range(KH):
                        nc.tensor.matmul(
                            pb2,
                            lhsT=htsb[:, kt, tt * 128 : (tt + 1) * 128],
                            rhs=w2sb["agg"][:, kt, :],
                            start=(kt == 0),
                            stop=(kt == KH - 1) and not with_b2,
                        )
                    if with_b2:
                        nc.tensor.matmul(
                            pb2, lhsT=ones_col, rhs=b2sb["agg"],
                            start=False, stop=True,
                        )
                    nc.scalar.activation(q_sb[:, tt, :], pb2, AF.Tanh)
                    tg = c * NTT + tt
                    nc.vector.tensor_tensor(
                        q2_sb[:, tt, :], q_sb[:, tt, :], q_sb[:, tt, :], OP.mult
                    )
                    # candidates on VectorE (per-partition AP scalars) so the
                    # scalar engine only owes the tanh per tile
                    cand = elu.tile([128, A], BF16, tag="cand", name="cand")
                    nc.vector.tensor_scalar(
                        cand, q_sb[:, tt, :],
                        mask_sb[:, s, tg : tg + 1], negpad_sb[:, s, tg : tg + 1],
                        OP.mult, OP.add,
                    )
                    nc.vector.tensor_tensor(
                        acc_max[:, s, :], acc_max[:, s, :], cand, OP.max
                    )
                    cand2 = elu.tile([128, A], BF16, tag="cand2", name="cand2")
                    nc.vector.tensor_scalar(
                        cand2, q_sb[:, tt, :],
                        negmask_sb[:, s, tg : tg + 1], negpad_sb[:, s, tg : tg + 1],
                        OP.mult, OP.add,
                    )
                    nc.vector.tensor_tensor(
                        acc_nmax[:, s, :], acc_nmax[:, s, :], cand2, OP.max
                    )
                return q_sb, q2_sb

            def emit_mm2_adj(s, c, htsb, tts=None):
                for tt in (tts if tts is not None else range(NTT)):
                    pb2 = pb2p.tile([128, A], F32, tag="pb2", name="pb2")
                    for kt in range(KH):
                        nc.tensor.matmul(
                            pb2,
                            lhsT=htsb[:, kt, tt * 128 : (tt + 1) * 128],
                            rhs=w2sb["adj"][:, kt, :],
                            start=(kt == 0),
                            stop=(kt == KH - 1) and not with_b2,
                        )
                    if with_b2:
                        nc.tensor.matmul(
                            pb2, lhsT=ones_col, rhs=b2sb["adj"],
                            start=False, stop=True,
                        )
                    adj_sb = adjo.tile([128, A], F32, tag="adj", name="adj")
                    nc.scalar.activation(adj_sb, pb2, AF.Tanh)
                    nc.sync.dma_start(
                        out=out[
                            s, 4 + c * 512 + tt * 128 : 4 + c * 512 + (tt + 1) * 128, :
                        ],
                        in_=adj_sb,
                    )

            # ---- finalize closures ----------------------------------------
            # part 1 (fold + mean + var chain + fin of the last bag) can run
            # under the last chunk's adj matmuls; part 2 (sqrt + std rows)
            # must follow the last tanh so the ACT table only swaps once
            fstate = {}

            def emit_finalize_pre():
                folds = {}
                for nm, psum in (("s", psum_s), ("q", psum_q)):
                    cp = fin.tile([128, A], F32R, tag="scp", name=f"scp_{nm}", bufs=2)
                    nc.scalar.copy(cp, psum)
                    pf = pb2p.tile([BPC, A], F32, tag="pb2", name=f"pf_{nm}")
                    nc.tensor.matmul(
                        pf, lhsT=sel_sb, rhs=cp, start=True, stop=True,
                        skip_group_check=True,
                    )
                    folds[nm] = pf
                mean_sb = fin.tile([BPC, A], F32, tag="frow", name="mean", bufs=2)
                nc.vector.tensor_scalar(
                    mean_sb, folds["s"], invn_sb[:, 0:1], None, OP.mult
                )
                m2_sb = fin.tile([BPC, A], F32, tag="fch", name="m2", bufs=4)
                nc.vector.tensor_tensor(m2_sb, mean_sb, mean_sb, OP.mult)
                s1_sb = fin.tile([BPC, A], F32, tag="fch", name="s1", bufs=4)
                nc.vector.tensor_scalar(
                    s1_sb, folds["q"], invnm1_sb[:, 0:1], None, OP.mult
                )
                s2_sb = fin.tile([BPC, A], F32, tag="fch", name="s2", bufs=4)
                nc.vector.tensor_scalar(
                    s2_sb, m2_sb, novernm1_sb[:, 0:1], None, OP.mult
                )
                var_sb = fin.tile([BPC, A], F32, tag="fch", name="var", bufs=4)
                nc.vector.tensor_tensor(var_sb, s1_sb, s2_sb, OP.subtract)
                for s in range(BPC):
                    nc.scalar.dma_start(out=out[s, 0:1, :], in_=mean_sb[s : s + 1, :])
                emit_fin(order[BPC - 1], last=True)
                fstate["var"] = var_sb

            def emit_finalize_post():
                var_sb = fstate["var"]
                std_sb = fin.tile([BPC, A], F32, tag="frow", name="std", bufs=2)
                nc.scalar.activation(std_sb, var_sb, AF.Sqrt)
                for s in range(BPC):
                    nc.scalar.dma_start(out=out[s, 3:4, :], in_=std_sb[s : s + 1, :])

            # ---- main loops ------------------------------------------------
            pending_fin = []
            seq = [0]
            last_agg = (order[BPC - 1], len(widths[order[BPC - 1]]) - 1)

            def flush_stats():
                while pending_stats and pending_stats[0][0] < seq[0]:
                    emit_stats(pending_stats.pop(0))

            def queue_stats(s, c, q_sb, q2_sb, ntl):
                if (s, c) == last_agg:
                    while pending_stats:
                        emit_stats(pending_stats.pop(0))
                    emit_stats((seq[0], s, c, q_sb, q2_sb, ntl))
                else:
                    pending_stats.append((seq[0], s, c, q_sb, q2_sb, ntl))

            # depth-2 x prefetch: the load for chunk i+2 is issued while
            # chunk i runs, so transfers never gate the next chunk's mm1
            xtiles = {(s0, 0): xb00, (s0, 1): xb01}
            pf_list = [
                (order[bi], c) for bi in range(BPC) for c in range(NCH)
            ][2:]
            pf_idx = [0]

            def xb_prefetch(n=1):
                for _ in range(n):
                    if pf_idx[0] >= len(pf_list):
                        return
                    s_, c_ = pf_list[pf_idx[0]]
                    pf_idx[0] += 1
                    t = xin.tile([128, KE, 512], F32R, tag="xb", name=f"xb{s_}{c_}")
                    nc.sync.dma_start(out=t, in_=xt[s_, c_])
                    xtiles[(s_, c_)] = t

            for bi in range(BPC):
                s = order[bi]
                wlist = widths[s]
                if bi == 0 and split_bag0:
                    # agg-only phases for chunks 0-1, then their adj phases:
                    # keeps the early weight/x demand inside the two HWDGE
                    # queues' delivery rate
                    xbs = [xtiles[(s, 0)], xtiles[(s, 1)]]
                    hts_agg = []
                    for c in (0, 1):
                        hts_agg.append(emit_mm1(s, c, "agg", xbs[c], wlist[c]))
                        xb_prefetch()
                        seq[0] += 1
                    for c in (0, 1):
                        ntl = max(0, min(NTT, tct[s] - NTT * c))
                        q_sb, q2_sb = emit_mm2_agg(s, c, hts_agg[c], ntl)
                        queue_stats(s, c, q_sb, q2_sb, ntl)
                        flush_stats()
                        seq[0] += 1
                    for c in (0, 1):
                        ht_adj = emit_mm1(s, c, "adj", xbs[c], 512)
                        emit_mm2_adj(s, c, ht_adj)
                        flush_stats()
                        seq[0] += 1
                    start_c = 2
                else:
                    start_c = 0
                for c in range(start_c, NCH):
                    w = wlist[c] if c < len(wlist) else 0
                    ntl = max(0, min(NTT, tct[s] - NTT * c))
                    xb_prefetch()
                    xb = xtiles[(s, c)]
                    hts = {}
                    for m in (("agg", "adj") if w else ("adj",)):
                        hts[m] = emit_mm1(s, c, m, xb, w if m == "agg" else 512)
                    if w:
                        q_sb, q2_sb = emit_mm2_agg(s, c, hts["agg"], ntl)
                        queue_stats(s, c, q_sb, q2_sb, ntl)
                    if (
                        bi == BPC - 1 and c == NCH - 1 and not w
                        and not pending_stats and pos_cnt_s == pos_total
                    ):
                        # two adj tiles first so ScalarE gets ahead with the
                        # fold inputs, two after to cover the fin chain
                        emit_mm2_adj(s, c, hts["adj"], tts=(0, 1))
                        emit_finalize_pre()
                        emit_mm2_adj(s, c, hts["adj"], tts=(2, 3))
                    else:
                        emit_mm2_adj(s, c, hts["adj"])
                    flush_stats()
                    if c == 1 and pending_fin:
                        emit_fin(pending_fin.pop(0))
                    seq[0] += 1
                if bi < BPC - 1:
                    pending_fin.append(s)

            while pending_stats:
                emit_stats(pending_stats.pop(0))
            assert pos_cnt_s == pos_total and pos_cnt_q == pos_total

            if "var" not in fstate:
                emit_finalize_pre()
            emit_finalize_post()
    _split_waits(nc)
    return nc


_PROGRAM_CACHE: dict = {}


def kernel(**inputs) -> np.ndarray:
    x = np.asarray(inputs["x"], np.float32)
    lengths = np.asarray(inputs["padding_lengths"]).astype(np.int64)
    agg_W1 = np.asarray(inputs["agg_W1"], np.float32)
    agg_b1 = np.asarray(inputs["agg_b1"], np.float32)
    agg_W2 = np.asarray(inputs["agg_W2"], np.float32)
    agg_b2 = np.asarray(inputs["agg_b2"], np.float32)
    adj_W1 = np.asarray(inputs["adj_W1"], np.float32)
    adj_b1 = np.asarray(inputs["adj_b1"], np.float32)
    adj_W2 = np.asarray(inputs["adj_W2"], np.float32)
    adj_b2 = np.asarray(inputs["adj_b2"], np.float32)

    with_b2 = bool(np.any(agg_b2) or np.any(adj_b2))
    b1_zero = not (np.any(agg_b1) or np.any(adj_b1))

    # cluster bags by length so short bags share a slot across cores and the
    # agg path processes only ceil(slot_max/128) token tiles per slot
    perm = np.argsort(lengths, kind="stable")  # slot s holds ranks [8s, 8s+8)
    tct = tuple(
        int(min(16, max(1, np.ceil(lengths[perm[s * NCORES : (s + 1) * NCORES]].max() / 128))))
        for s in range(BPC)
    )
    order = tuple(sorted(range(BPC), key=lambda s: -tct[s]))  # longest first
    key = (with_b2, b1_zero, tct)
    if key not in _PROGRAM_CACHE:
        _PROGRAM_CACHE[key] = _build_program(with_b2, b1_zero, tct, order)
    nc = _PROGRAM_CACHE[key]

    # ---- host-side input prep (pre-tiled to SBUF layouts) -----------------
    # xt[b, c, p, kt, t] = x[b, c*512 + t, kt*128 + p]
    xt = np.ascontiguousarray(
        x.reshape(B, NCH, 512, KE, 128).transpose(0, 1, 4, 3, 2)
    )

    def tile_w1(w):  # [H, E] -> [128, NHT, KE, 128]; [p, ht, kt, h']
        wt = w.T.reshape(KE, 128, NHT, 128).transpose(1, 2, 0, 3)
        return np.ascontiguousarray(wt)

    def tile_w2(w):  # [A, H] -> [128, KH, A]; [p, kt, a]
        wt = w.T.reshape(KH, 128, A).transpose(1, 0, 2)
        return np.ascontiguousarray(wt)

    w1t = {"agg": tile_w1(agg_W1), "adj": tile_w1(adj_W1)}
    w2t = {"agg": tile_w2(agg_W2), "adj": tile_w2(adj_W2)}
    b1 = {
        "agg": np.ascontiguousarray(agg_b1.reshape(NHT, 128).T),
        "adj": np.ascontiguousarray(adj_b1.reshape(NHT, 128).T),
    }
    b2 = {"agg": agg_b2.reshape(1, A), "adj": adj_b2.reshape(1, A)}

    mask = (np.arange(T)[None, :] < lengths[:, None]).astype(np.float32)  # [B, T]
    negmask = -mask
    negpad = (mask - 1.0) * BIG
    # stationary mask columns for the stats matmuls: [B, 16 tok-tiles, 128, BPC]
    slhs_local = np.zeros((B, 16, 128, BPC), np.float32)
    mask_t = mask.reshape(B, 16, 128)
    rank_of = np.empty(B, np.int64)
    rank_of[perm] = np.arange(B)
    for b in range(B):
        slhs_local[b, :, :, int(rank_of[b]) // NCORES] = mask_t[b]
    # fold-matmul selector: sums psum partitions {32j + b} into row b
    sel4 = np.zeros((128, BPC), np.float32)
    for j in range(NTT):
        for b in range(BPC):
            sel4[32 * j + b, b] = 1.0
    n = lengths.astype(np.float64)
    inv_n = (1.0 / n).astype(np.float32).reshape(B, 1)
    inv_nm1 = (1.0 / (n - 1.0)).astype(np.float32).reshape(B, 1)
    n_over_nm1 = (n / (n - 1.0)).astype(np.float32).reshape(B, 1)

    def to_bf16(a):
        import jax.numpy as jnp
        return np.asarray(jnp.asarray(a, jnp.bfloat16))

    identb = to_bf16(np.eye(128, dtype=np.float32))

    in_maps = []
    for c in range(NCORES):
        sl = perm[c::NCORES]  # slot s of core c = perm[s * NCORES + c]
        im = {
            "xt": xt[sl],
            "w1t_agg": w1t["agg"], "w2t_agg": w2t["agg"],
            "w1t_adj": w1t["adj"], "w2t_adj": w2t["adj"],
            "mask": np.ascontiguousarray(
                mask[sl].reshape(BPC, 16, 128).transpose(2, 0, 1)
            ),
            "negmask": np.ascontiguousarray(
                negmask[sl].reshape(BPC, 16, 128).transpose(2, 0, 1)
            ),
            "negpad": np.ascontiguousarray(
                negpad[sl].reshape(BPC, 16, 128).transpose(2, 0, 1)
            ),
            "slhs": to_bf16(slhs_local[sl].transpose(2, 0, 1, 3)),
            "sel4": sel4,
            "identb": identb,
            "inv_n": inv_n[sl], "inv_nm1": inv_nm1[sl],
            "n_over_nm1": n_over_nm1[sl],
        }
        if not b1_zero:
            im["b1_agg"] = b1["agg"]
            im["b1_adj"] = b1["adj"]
        if with_b2:
            im["b2_agg"] = b2["agg"]
            im["b2_adj"] = b2["adj"]
        in_maps.append(im)

    res = run_bass_kernel_spmd(nc, in_maps, core_ids=list(range(NCORES)))
    out = np.empty((B, 4 + T, A), np.float32)
    for c in range(NCORES):
        out[perm[c::NCORES]] = res.results[c]["out"]
    return out



# revision 2
# speedup vs baseline: 1.0870x; 1.0870x over previous
"""Trainium2 Bass kernel for nn_AggregateConcatenate.

out[b] = concat([masked {mean,max,min,std} of tanh-MLP_agg(x_b) over the valid
prefix, tanh-MLP_adj(x_b)], axis=1)

Sharding: data-parallel over batch B across 8 NeuronCores (4 bags/core).
Bags are clustered by length into 4 slots (8 similar-length bags share a
slot across cores); slots are processed longest-first so the kernel tail is
adj-only matmul work under which the ragged finalization drains.

Per-core dataflow (matmuls in float16: 1.0 cyc/row vs float32r 1.06):
  - mm1 feature-major: h^T[h, tok] = W1T.T @ x^T (x pre-transposed on host),
    clipped per-slot to ceil(valid/128) tokens (width >= 256 keeps full rate).
  - ELU via the exact identity elu(z) = min(exp(z) - 1, relu(z)):
    Exp and Relu on ScalarE (bias b1 fused), one scalar_tensor_tensor on
    VectorE. Keeps VectorE comfortably faster than the PE mm1 stream.
  - mm2 token-major: q[tok, a] = hT.T @ W2T -> adjacent output rows DMA out
    contiguously; the ragged reductions see tokens on partitions.
  - masked sum / sum-of-squares via bf16 PE matmuls with per-(bag, tile) mask
    columns stationary, col-tiled so the 4 token tiles of a chunk run in
    different PE column groups concurrently; folded at the end with one
    select-matmul. max/min via bf16 candidates folded into per-bag bf16
    accumulators on VectorE, transposed with regular pipelined bf16 matmuls
    against an identity and reduced on VectorE.
  - startup: the first bag runs agg-only phases for chunks 0-1 before their
    adj phases, so the early PE demand (w1_agg + x + w2_agg = 6MB over 40us)
    stays within what the two HWDGE queues deliver; weights stream in
    first-use-order chunks while the PE warms the HAM clock gate on junk
    matmuls.
"""

import numpy as np

import concourse.bass as bass
import concourse.tile as tile
from concourse import mybir
from concourse.bass_utils import run_bass_kernel_spmd
from concourse.tile import ScopedClock

B, T, E, H, A = 32, 2048, 512, 1024, 512
NCORES = 8
BPC = B // NCORES          # bags per core (= slots)
NCH = T // 512             # 512-token chunks per bag
NTT = 4                    # 128-token tiles per chunk
KE = E // 128              # k-tiles for mm1
KH = H // 128              # k-tiles for mm2
NHT = H // 128             # h-tiles (partition tiles of H)
BIG = 30.0                 # ragged padding offset; |tanh| < 1 << BIG

F32 = mybir.dt.float32
F32R = mybir.dt.float32r
BF16 = mybir.dt.bfloat16
F16 = mybir.dt.float16
AF = mybir.ActivationFunctionType
OP = mybir.AluOpType


class _SplitDrainTileContext(tile.TileContext):
    """TileContext whose exit drain splits sem waits across sync NOPs."""

    WAIT_LIMIT = 1

    def _drain_and_barrier(self, tick_clock, wait_clock):
        drain_inst = self.nc.sync.drain()
        wait_clock.add_sem_waits(
            drain_inst.ins, ScopedClock({None: tick_clock.global_clock})
        )
        si = drain_inst.ins.sync_info
        if si is not None and len(si.on_wait) > self.WAIT_LIMIT:
            waits = list(si.on_wait)
            drain_inst.ins.sync_info = mybir.SyncInfo(
                on_wait=waits[: self.WAIT_LIMIT], on_update=list(si.on_update)
            )
            for i in range(self.WAIT_LIMIT, len(waits), self.WAIT_LIMIT):
                nop = self.nc.sync.nop()
                nop.ins.sync_info = mybir.SyncInfo(
                    on_wait=waits[i : i + self.WAIT_LIMIT], on_update=[]
                )
        self.nc.all_engine_barrier()
        assert self.sems is not None
        popped = self.nc._tile_sem_poison_stack.pop()
        assert popped is self._sem_poison
        self.nc.clear_and_free_semaphores(list(self.sems.allocated().values()))
        self.nc.all_engine_barrier()


def _split_waits(nc, limit: int = 1):
    """Walrus codegen accepts at most one sync wait per TPB instruction.

    Hoist excess waits from any instruction onto injected same-engine NOPs
    placed immediately before it (same-engine program order is preserved, so
    waiting earlier is equivalent).
    """
    uid = [0]
    for f in nc.m.functions:
        for bb in f.blocks:
            new_insts = []
            for ins in bb.instructions:
                si = ins.sync_info
                if si is not None and len(si.on_wait) > limit:
                    waits = list(si.on_wait)
                    keep = waits[:limit]
                    rest = waits[limit:]
                    for j in range(0, len(rest), limit):
                        uid[0] += 1
                        nop = mybir.InstNoOp(
                            name=f"waitnop-{uid[0]}",
                            engine=ins.engine,
                            ins=[],
                            outs=[],
                        )
                        nop.sync_info = mybir.SyncInfo(
                            on_wait=rest[j : j + limit], on_update=[]
                        )
                        new_insts.append(nop)
                    ins.sync_info = mybir.SyncInfo(
                        on_wait=keep, on_update=list(si.on_update)
                    )
                new_insts.append(ins)
            if len(new_insts) != len(bb.instructions):
                bb.instructions = new_insts
    return nc


def _widths(tct: int) -> list:
    """mm1 chunk widths covering ceil(tct) 128-token tiles, each >= 256."""
    full, rem = divmod(tct, NTT)
    w = [512] * full
    if rem:
        w.append(max(256, rem * 128))
    return w


def _build_program(with_b2: bool, b1_zero: bool, tct: tuple, order: tuple):
    nc = bass.Bass()

    # all tensors arrive pre-tiled to the exact SBUF layout so every DMA is
    # a contiguous max-line-size copy
    xt = nc.declare_dram_parameter("xt", [BPC, NCH, 128, KE, 512], F16, isOutput=False)
    w1t = {}
    w2t = {}
    b1 = {}
    b2 = {}
    for m in ("agg", "adj"):
        w1t[m] = nc.declare_dram_parameter(
            f"w1t_{m}", [128, NHT, KE, 128], F16, isOutput=False
        )
        w2t[m] = nc.declare_dram_parameter(
            f"w2t_{m}", [128, KH, A], F16, isOutput=False
        )
        if not b1_zero:
            b1[m] = nc.declare_dram_parameter(f"b1_{m}", [128, NHT], F32, isOutput=False)
        if with_b2:
            b2[m] = nc.declare_dram_parameter(f"b2_{m}", [1, A], F32R, isOutput=False)
    maskd = nc.declare_dram_parameter("mask", [128, BPC, 16], F32, isOutput=False)
    negmaskd = nc.declare_dram_parameter("negmask", [128, BPC, 16], F32, isOutput=False)
    negpadd = nc.declare_dram_parameter("negpad", [128, BPC, 16], F32, isOutput=False)
    slhsd = nc.declare_dram_parameter("slhs", [128, BPC, 16, BPC], F16, isOutput=False)
    seld = nc.declare_dram_parameter("sel4", [128, BPC], F32R, isOutput=False)
    identd = nc.declare_dram_parameter("identb", [128, 128], F16, isOutput=False)
    invnd = nc.declare_dram_parameter("inv_n", [BPC, 1], F32, isOutput=False)
    invnm1d = nc.declare_dram_parameter("inv_nm1", [BPC, 1], F32, isOutput=False)
    novernm1d = nc.declare_dram_parameter("n_over_nm1", [BPC, 1], F32, isOutput=False)
    out = nc.declare_dram_parameter("out", [BPC, 4 + T, A], F32, isOutput=True)

    widths = {s: _widths(tct[s]) for s in range(BPC)}
    # stat matmuls per col-group position j (tile index within chunk)
    pos_total = [0] * NTT
    for s in range(BPC):
        for j in range(NTT):
            pos_total[j] += max(0, -(-(tct[s] - j) // NTT)) if tct[s] > j else 0

    s0 = order[0]
    split_bag0 = len(widths[s0]) >= 2

    with _SplitDrainTileContext(nc) as tc:
        with (
            tc.tile_pool(name="consts", bufs=1) as consts,
            tc.tile_pool(name="accs", bufs=1) as accs,
            tc.tile_pool(name="xin", bufs=(3 if split_bag0 else 2)) as xin,
            tc.tile_pool(name="elu", bufs=2) as elu,
            tc.tile_pool(name="ht", bufs=1) as htp,
            tc.tile_pool(name="qp", bufs=1) as qp,
            tc.tile_pool(name="adjo", bufs=2) as adjo,
            tc.tile_pool(name="fin", bufs=1) as fin,
            tc.tile_pool(name="pb1", bufs=(2 if b1_zero else 4), space="PSUM") as pb1p,
            tc.tile_pool(name="pb2", bufs=2, space="PSUM") as pb2p,
            tc.tile_pool(name="pstat", bufs=1, space="PSUM") as pstat,
        ):
            # ---- DVE-side init first so the PE warmup can start ASAP ------
            junk = consts.tile([128, 512], F16, tag="junk", name="junk")
            nc.vector.memset(junk, 0.001)
            junk_r = junk
            warm_sb = consts.tile([1, 1], F32, tag="warm", name="warm")
            nc.vector.memset(warm_sb, 0.0)

            # first x chunk split by k-tiles for fastest time-to-first-matmul
            xb00 = xin.tile([128, KE, 512], F16, tag="xb", name="xb00")
            nc.sync.dma_start(out=xb00[:, 0:1, :], in_=xt[s0, 0, :, 0:1, :])
            nc.sync.dma_start(out=xb00[:, 1:2, :], in_=xt[s0, 0, :, 1:2, :])
            nc.sync.dma_start(out=xb00[:, 2:4, :], in_=xt[s0, 0, :, 2:4, :])

            # ---- weights: first-use-order chunks across both HWDGE queues -
            w1sb = {}
            w2sb = {}
            b1sb = {}
            b2sb = {}
            for m in ("agg", "adj"):
                w1sb[m] = consts.tile(
                    [128, NHT, KE, 128], F16, tag=f"w1_{m}", name=f"w1_{m}"
                )
                w2sb[m] = consts.tile([128, KH, A], F16, tag=f"w2_{m}", name=f"w2_{m}")
            # agg weights on the ACT queue (scalar engine is free this early);
            # chunk sizes track the mm1/mm2 consumption rate
            # h0-h3 on ACT; h4-h7 ride the SP queue right after the x
            # prefetch so both queues feed mm1_agg in parallel
            for h0 in range(4):
                nc.scalar.dma_start(
                    out=w1sb["agg"][:, h0 : h0 + 1, :, :],
                    in_=w1t["agg"][:, h0 : h0 + 1, :, :],
                )
            for h0, h1 in ((4, 6), (6, 8)):
                nc.sync.dma_start(
                    out=w1sb["agg"][:, h0:h1, :, :], in_=w1t["agg"][:, h0:h1, :, :]
                )
            # the second x chunk + remaining weights behind the prefetch,
            # with w2_agg split across both queues so neither serializes
            xb01 = xin.tile([128, KE, 512], F16, tag="xb", name="xb01")
            nc.sync.dma_start(out=xb01[:, 0:2, :], in_=xt[s0, 1, :, 0:2, :])
            nc.sync.dma_start(out=xb01[:, 2:4, :], in_=xt[s0, 1, :, 2:4, :])
            nc.scalar.dma_start(out=w2sb["agg"][:, 0:2, :], in_=w2t["agg"][:, 0:2, :])
            for k0, k1 in ((2, 4), (4, 6), (6, 8)):
                nc.sync.dma_start(
                    out=w2sb["agg"][:, k0:k1, :], in_=w2t["agg"][:, k0:k1, :]
                )
            for h0, h1 in ((0, 4), (4, 8)):
                nc.sync.dma_start(
                    out=w1sb["adj"][:, h0:h1, :, :], in_=w1t["adj"][:, h0:h1, :, :]
                )
            for k0, k1 in ((0, 4), (4, 8)):
                nc.sync.dma_start(
                    out=w2sb["adj"][:, k0:k1, :], in_=w2t["adj"][:, k0:k1, :]
                )

            # small consts on the SWDGE queue
            mask_sb = consts.tile([128, BPC, 16], F32, tag="mask", name="mask")
            nc.gpsimd.dma_start(out=mask_sb, in_=maskd[:, :, :])
            negmask_sb = consts.tile([128, BPC, 16], F32, tag="negmask", name="negmask")
            nc.gpsimd.dma_start(out=negmask_sb, in_=negmaskd[:, :, :])
            negpad_sb = consts.tile([128, BPC, 16], F32, tag="negpad", name="negpad")
            nc.gpsimd.dma_start(out=negpad_sb, in_=negpadd[:, :, :])
            slhs_sb = consts.tile([128, BPC, 16, BPC], F16, tag="slhs", name="slhs")
            nc.gpsimd.dma_start(out=slhs_sb, in_=slhsd[:, :, :, :])
            sel_sb = consts.tile([128, BPC], F32R, tag="sel4", name="sel4")
            nc.gpsimd.dma_start(out=sel_sb, in_=seld[:, :])
            ident_b = consts.tile([128, 128], F16, tag="identb", name="identb")
            nc.gpsimd.dma_start(out=ident_b, in_=identd[:, :])
            invn_sb = consts.tile([BPC, 1], F32, tag="invn", name="invn")
            nc.gpsimd.dma_start(out=invn_sb, in_=invnd[:, :])
            invnm1_sb = consts.tile([BPC, 1], F32, tag="invnm1", name="invnm1")
            nc.gpsimd.dma_start(out=invnm1_sb, in_=invnm1d[:, :])
            novernm1_sb = consts.tile([BPC, 1], F32, tag="novernm1", name="novernm1")
            nc.gpsimd.dma_start(out=novernm1_sb, in_=novernm1d[:, :])
            for m in ("agg", "adj"):
                if not b1_zero:
                    b1sb[m] = consts.tile([128, NHT], F32, tag=f"b1_{m}", name=f"b1_{m}")
                    nc.gpsimd.dma_start(out=b1sb[m], in_=b1[m][:, :])
                if with_b2:
                    b2sb[m] = consts.tile([1, A], F32R, tag=f"b2_{m}", name=f"b2_{m}")
                    nc.gpsimd.dma_start(out=b2sb[m], in_=b2[m][:, :])
            if with_b2:
                ones_col = consts.tile([1, 128], F32R, tag="ones", name="ones")
                nc.gpsimd.memset(ones_col, 1.0)

            # per-bag running accumulators for max(q) and max(-q), bf16 so
            # the final transposes run as full-rate bf16 matmuls
            acc_max = accs.tile([128, BPC, A], F16, tag="acc_max", name="acc_max")
            nc.gpsimd.memset(acc_max, -2.0)
            acc_nmax = accs.tile([128, BPC, A], F16, tag="acc_nmax", name="acc_nmax")
            nc.gpsimd.memset(acc_nmax, -2.0)

            # pre-warm the ACT exp table set during the DMA preamble
            nc.scalar.activation(warm_sb, warm_sb, AF.Exp)

            # warm the PE HAM clock gate while the first weights stream in
            # 14 junk matmuls: >3.4us of sustained PE activity trips the HAM
            # clock gate to full speed, and they productively burn the
            # window where the first weight chunks are still in flight
            pwarm = pb2p.tile([128, A], F32, tag="pb2", name="pwarm")
            for _ in range(14):
                nc.tensor.matmul(
                    pwarm, lhsT=junk_r[:, 0:128], rhs=junk_r, start=True, stop=True
                )

            # stats accumulators in PSUM: col-group j (tile-in-chunk) keeps
            # its partial sums at partitions 32j+bag; memset so the fold
            # copy never reads uninitialized PSUM on unused partitions
            psum_s = pstat.tile([128, A], F32, tag="psum_s", name="psum_s")
            nc.vector.memset(psum_s, 0.0)
            psum_q = pstat.tile([128, A], F32, tag="psum_q", name="psum_q")
            nc.vector.memset(psum_q, 0.0)

            # ---- emitters --------------------------------------------------
            pending_stats = []
            pos_cnt_s = [0] * NTT
            pos_cnt_q = [0] * NTT
            hgrp = 2 if b1_zero else 1

            def emit_stats(item):
                _, s, c, pq, pq2, ntl = item
                for psum, pqx, cnts in (
                    (psum_s, pq, pos_cnt_s), (psum_q, pq2, pos_cnt_q)
                ):
                    for tt in range(ntl):
                        tg = c * NTT + tt
                        lhs = slhs_sb[:, s, tg, :]
                        tp = (0, 32 * tt) if tt else None
                        nc.tensor.matmul(
                            psum[32 * tt : 32 * tt + BPC, :], lhsT=lhs,
                            rhs=pqx[:, tt, :],
                            start=(cnts[tt] == 0), stop=(cnts[tt] == pos_total[tt] - 1),
                            tile_position=tp, skip_group_check=True,
                        )
                        cnts[tt] += 1

            def emit_fin(s, last=False):
                # cross-partition max: transpose 128x128 blocks of the bf16
                # accumulators with regular pipelined matmuls against an
                # identity, reduce over the free axis on VectorE
                for acc, row, neg in ((acc_max, 1, False), (acc_nmax, 2, True)):
                    redt = fin.tile(
                        [128, NTT], F16, tag="redt", name=f"redt_{s}_{row}", bufs=2
                    )
                    for ch in range(NTT):
                        # one PSUM-ring tile per transpose so consecutive
                        # transposes alternate banks instead of ping-ponging
                        # against the reduce on a single bank; the last fin
                        # also borrows the then-idle pb1 ring for 4-deep
                        if last and ch % 2:
                            pt = pb1p.tile([128, 128], F32, tag="pb1", name="pt_fin")
                        else:
                            pt = pb2p.tile([128, 128], F32, tag="pb2", name="pt_fin")
                        nc.tensor.matmul(
                            pt, lhsT=acc[:, s, ch * 128 : (ch + 1) * 128],
                            rhs=ident_b, start=True, stop=True,
                            skip_group_check=True,
                        )
                        nc.vector.tensor_reduce(
                            redt[:, ch : ch + 1], pt,
                            axis=mybir.AxisListType.X, op=OP.max,
                        )
                    prow = pb2p.tile([NTT, 128], F32, tag="pb2", name="prow_fin")
                    nc.tensor.matmul(
                        prow, lhsT=redt, rhs=ident_b, start=True, stop=True,
                        skip_group_check=True,
                    )
                    row_sb = fin.tile(
                        [NTT, 128], F32, tag="row", name=f"row_{s}_{row}", bufs=2
                    )
                    nc.scalar.mul(row_sb, prow, -1.0 if neg else 1.0)
                    nc.sync.dma_start(
                        out=out[s, row : row + 1, :].rearrange(
                            "o (c f) -> (o c) f", c=NTT
                        ),
                        in_=row_sb,
                    )

            def emit_mm1(s, c, m, xb, wm):
                htsb = htp.tile(
                    [128, KH, 512], F16, tag=f"ht_{m}", name=f"ht_{m}",
                    bufs=(2 if (split_bag0 and m == "agg") else 1),
                )
                for hg in range(NHT // hgrp):
                    pb1 = pb1p.tile([128, hgrp, 512], F32, tag="pb1", name="pb1")
                    for j in range(hgrp):
                        ht = hg * hgrp + j
                        for kt in range(KE):
                            nc.tensor.matmul(
                                pb1[:, j, :wm],
                                lhsT=w1sb[m][:, ht, kt, :],
                                rhs=xb[:, kt, :wm],
                                start=(kt == 0),
                                stop=(kt == KE - 1),
                            )
                    bias = 0.0 if b1_zero else b1sb[m][:, hg : hg + 1]
                    pb1s = pb1[:, :, :wm]
                    e_sb = elu.tile([128, hgrp, 512], F32, tag="e", name="e")
                    nc.scalar.activation(e_sb[:, :, :wm], pb1s, AF.Exp, bias=bias)
                    r_sb = elu.tile([128, hgrp, 512], F32, tag="r", name="r")
                    nc.scalar.activation(r_sb[:, :, :wm], pb1s, AF.Relu, bias=bias)
                    nc.vector.scalar_tensor_tensor(
                        out=htsb[:, hg * hgrp : (hg + 1) * hgrp, :wm],
                        in0=e_sb[:, :, :wm], scalar=-1.0, in1=r_sb[:, :, :wm],
                        op0=OP.add, op1=OP.min,
                    )
                return htsb

            def emit_mm2_agg(s, c, htsb, ntl):
                q_sb = qp.tile([128, NTT, A], F16, tag="q", name="q")
                q2_sb = qp.tile([128, NTT, A], F16, tag="q2", name="q2")
                for tt in range(ntl):
                    pb2 = pb2p.tile([128, A], F32, tag="pb2", name="pb2")
                    for kt in range(KH):
                        nc.tensor.matmul(
                            pb2,
                            lhsT=htsb[:, kt, tt * 128 : (tt + 1) * 128],
                            rhs=w2sb["agg"][:, kt, :],
                            start=(kt == 0),
                            stop=(kt == KH - 1) and not with_b2,
                        )
                    if with_b2:
                        nc.tensor.matmul(
                            pb2, lhsT=ones_col, rhs=b2sb["agg"],
                            start=False, stop=True,
                        )
                    nc.scalar.activation(q_sb[:, tt, :], pb2, AF.Tanh)
                    tg = c * NTT + tt
                    nc.vector.tensor_tensor(
                        q2_sb[:, tt, :], q_sb[:, tt, :], q_sb[:, tt, :], OP.mult
                    )
                    # candidates on VectorE (per-partition AP scalars) so the
                    # scalar engine only owes the tanh per tile
                    cand = elu.tile([128, A], F16, tag="cand", name="cand")
                    nc.vector.tensor_scalar(
                        cand, q_sb[:, tt, :],
                        mask_sb[:, s, tg : tg + 1], negpad_sb[:, s, tg : tg + 1],
                        OP.mult, OP.add,
                    )
                    nc.vector.tensor_tensor(
                        acc_max[:, s, :], acc_max[:, s, :], cand, OP.max
                    )
                    cand2 = elu.tile([128, A], F16, tag="cand2", name="cand2")
                    nc.vector.tensor_scalar(
                        cand2, q_sb[:, tt, :],
                        negmask_sb[:, s, tg : tg + 1], negpad_sb[:, s, tg : tg + 1],
                        OP.mult, OP.add,
                    )
                    nc.vector.tensor_tensor(
                        acc_nmax[:, s, :], acc_nmax[:, s, :], cand2, OP.max
                    )
                return q_sb, q2_sb

            def emit_mm2_adj(s, c, htsb, tts=None):
                for tt in (tts if tts is not None else range(NTT)):
                    pb2 = pb2p.tile([128, A], F32, tag="pb2", name="pb2")
                    for kt in range(KH):
                        nc.tensor.matmul(
                            pb2,
                            lhsT=htsb[:, kt, tt * 128 : (tt + 1) * 128],
                            rhs=w2sb["adj"][:, kt, :],
                            start=(kt == 0),
                            stop=(kt == KH - 1) and not with_b2,
                        )
                    if with_b2:
                        nc.tensor.matmul(
                            pb2, lhsT=ones_col, rhs=b2sb["adj"],
                            start=False, stop=True,
                        )
                    adj_sb = adjo.tile([128, A], F32, tag="adj", name="adj")
                    nc.scalar.activation(adj_sb, pb2, AF.Tanh)
                    nc.sync.dma_start(
                        out=out[
                            s, 4 + c * 512 + tt * 128 : 4 + c * 512 + (tt + 1) * 128, :
                        ],
                        in_=adj_sb,
                    )

            # ---- finalize closures ----------------------------------------
            # part 1 (fold + mean + var chain + fin of the last bag) can run
            # under the last chunk's adj matmuls; part 2 (sqrt + std rows)
            # must follow the last tanh so the ACT table only swaps once
            fstate = {}

            def emit_finalize_pre():
                folds = {}
                for nm, psum in (("s", psum_s), ("q", psum_q)):
                    cp = fin.tile([128, A], F32R, tag="scp", name=f"scp_{nm}", bufs=2)
                    nc.scalar.copy(cp, psum)
                    pf = pb2p.tile([BPC, A], F32, tag="pb2", name=f"pf_{nm}")
                    nc.tensor.matmul(
                        pf, lhsT=sel_sb, rhs=cp, start=True, stop=True,
                        skip_group_check=True,
                    )
                    folds[nm] = pf
                mean_sb = fin.tile([BPC, A], F32, tag="frow", name="mean", bufs=2)
                nc.vector.tensor_scalar(
                    mean_sb, folds["s"], invn_sb[:, 0:1], None, OP.mult
                )
                m2_sb = fin.tile([BPC, A], F32, tag="fch", name="m2", bufs=4)
                nc.vector.tensor_tensor(m2_sb, mean_sb, mean_sb, OP.mult)
                s1_sb = fin.tile([BPC, A], F32, tag="fch", name="s1", bufs=4)
                nc.vector.tensor_scalar(
                    s1_sb, folds["q"], invnm1_sb[:, 0:1], None, OP.mult
                )
                s2_sb = fin.tile([BPC, A], F32, tag="fch", name="s2", bufs=4)
                nc.vector.tensor_scalar(
                    s2_sb, m2_sb, novernm1_sb[:, 0:1], None, OP.mult
                )
                var_sb = fin.tile([BPC, A], F32, tag="fch", name="var", bufs=4)
                nc.vector.tensor_tensor(var_sb, s1_sb, s2_sb, OP.subtract)
                for s in range(BPC):
                    nc.scalar.dma_start(out=out[s, 0:1, :], in_=mean_sb[s : s + 1, :])
                emit_fin(order[BPC - 1], last=True)
                fstate["var"] = var_sb

            def emit_finalize_post():
                var_sb = fstate["var"]
                std_sb = fin.tile([BPC, A], F32, tag="frow", name="std", bufs=2)
                nc.scalar.activation(std_sb, var_sb, AF.Sqrt)
                for s in range(BPC):
                    nc.scalar.dma_start(out=out[s, 3:4, :], in_=std_sb[s : s + 1, :])

            # ---- main loops ------------------------------------------------
            pending_fin = []
            seq = [0]
            last_agg = (order[BPC - 1], len(widths[order[BPC - 1]]) - 1)

            def flush_stats():
                while pending_stats and pending_stats[0][0] < seq[0]:
                    emit_stats(pending_stats.pop(0))

            def queue_stats(s, c, q_sb, q2_sb, ntl):
                if (s, c) == last_agg:
                    while pending_stats:
                        emit_stats(pending_stats.pop(0))
                    emit_stats((seq[0], s, c, q_sb, q2_sb, ntl))
                else:
                    pending_stats.append((seq[0], s, c, q_sb, q2_sb, ntl))

            # depth-2 x prefetch: the load for chunk i+2 is issued while
            # chunk i runs, so transfers never gate the next chunk's mm1
            xtiles = {(s0, 0): xb00, (s0, 1): xb01}
            pf_list = [
                (order[bi], c) for bi in range(BPC) for c in range(NCH)
            ][2:]
            pf_idx = [0]

            def xb_prefetch(n=1):
                for _ in range(n):
                    if pf_idx[0] >= len(pf_list):
                        return
                    s_, c_ = pf_list[pf_idx[0]]
                    pf_idx[0] += 1
                    t = xin.tile([128, KE, 512], F16, tag="xb", name=f"xb{s_}{c_}")
                    nc.sync.dma_start(out=t, in_=xt[s_, c_])
                    xtiles[(s_, c_)] = t

            for bi in range(BPC):
                s = order[bi]
                wlist = widths[s]
                if bi == 0 and split_bag0:
                    # agg-only phases for chunks 0-1, then their adj phases:
                    # keeps the early weight/x demand inside the two HWDGE
                    # queues' delivery rate
                    xbs = [xtiles[(s, 0)], xtiles[(s, 1)]]
                    hts_agg = []
                    for c in (0, 1):
                        hts_agg.append(emit_mm1(s, c, "agg", xbs[c], wlist[c]))
                        xb_prefetch()
                        seq[0] += 1
                    for c in (0, 1):
                        ntl = max(0, min(NTT, tct[s] - NTT * c))
                        q_sb, q2_sb = emit_mm2_agg(s, c, hts_agg[c], ntl)
                        queue_stats(s, c, q_sb, q2_sb, ntl)
                        flush_stats()
                        seq[0] += 1
                    for c in (0, 1):
                        ht_adj = emit_mm1(s, c, "adj", xbs[c], 512)
                        emit_mm2_adj(s, c, ht_adj)
                        flush_stats()
                        seq[0] += 1
                    start_c = 2
                else:
                    start_c = 0
                for c in range(start_c, NCH):
                    w = wlist[c] if c < len(wlist) else 0
                    ntl = max(0, min(NTT, tct[s] - NTT * c))
                    xb_prefetch()
                    xb = xtiles[(s, c)]
                    hts = {}
                    for m in (("agg", "adj") if w else ("adj",)):
                        hts[m] = emit_mm1(s, c, m, xb, w if m == "agg" else 512)
                    if w:
                        q_sb, q2_sb = emit_mm2_agg(s, c, hts["agg"], ntl)
                        queue_stats(s, c, q_sb, q2_sb, ntl)
                    if (
                        bi == BPC - 1 and c == NCH - 1 and not w
                        and not pending_stats and pos_cnt_s == pos_total
                    ):
                        # two adj tiles first so ScalarE gets ahead with the
                        # fold inputs, two after to cover the fin chain
                        emit_mm2_adj(s, c, hts["adj"], tts=(0, 1))
                        emit_finalize_pre()
                        emit_mm2_adj(s, c, hts["adj"], tts=(2, 3))
                    else:
                        emit_mm2_adj(s, c, hts["adj"])
                    flush_stats()
                    if c == 1 and pending_fin:
                        emit_fin(pending_fin.pop(0))
                    seq[0] += 1
                if bi < BPC - 1:
                    pending_fin.append(s)

            while pending_stats:
                emit_stats(pending_stats.pop(0))
            assert pos_cnt_s == pos_total and pos_cnt_q == pos_total

            if "var" not in fstate:
                emit_finalize_pre()
            emit_finalize_post()
    _split_waits(nc)
    return nc


_PROGRAM_CACHE: dict = {}


def kernel(**inputs) -> np.ndarray:
    x = np.asarray(inputs["x"], np.float32)
    lengths = np.asarray(inputs["padding_lengths"]).astype(np.int64)
    agg_W1 = np.asarray(inputs["agg_W1"], np.float32)
    agg_b1 = np.asarray(inputs["agg_b1"], np.float32)
    agg_W2 = np.asarray(inputs["agg_W2"], np.float32)
    agg_b2 = np.asarray(inputs["agg_b2"], np.float32)
    adj_W1 = np.asarray(inputs["adj_W1"], np.float32)
    adj_b1 = np.asarray(inputs["adj_b1"], np.float32)
    adj_W2 = np.asarray(inputs["adj_W2"], np.float32)
    adj_b2 = np.asarray(inputs["adj_b2"], np.float32)

    with_b2 = bool(np.any(agg_b2) or np.any(adj_b2))
    b1_zero = not (np.any(agg_b1) or np.any(adj_b1))

    # cluster bags by length so short bags share a slot across cores and the
    # agg path processes only ceil(slot_max/128) token tiles per slot
    perm = np.argsort(lengths, kind="stable")  # slot s holds ranks [8s, 8s+8)
    tct = tuple(
        int(min(16, max(1, np.ceil(lengths[perm[s * NCORES : (s + 1) * NCORES]].max() / 128))))
        for s in range(BPC)
    )
    order = tuple(sorted(range(BPC), key=lambda s: -tct[s]))  # longest first
    key = (with_b2, b1_zero, tct)
    if key not in _PROGRAM_CACHE:
        _PROGRAM_CACHE[key] = _build_program(with_b2, b1_zero, tct, order)
    nc = _PROGRAM_CACHE[key]

    # ---- host-side input prep (pre-tiled to SBUF layouts) -----------------
    # xt[b, c, p, kt, t] = x[b, c*512 + t, kt*128 + p]
    xt = np.ascontiguousarray(
        x.reshape(B, NCH, 512, KE, 128).transpose(0, 1, 4, 3, 2)
    ).astype(np.float16)

    def tile_w1(w):  # [H, E] -> [128, NHT, KE, 128]; [p, ht, kt, h']
        wt = w.T.reshape(KE, 128, NHT, 128).transpose(1, 2, 0, 3)
        return np.ascontiguousarray(wt).astype(np.float16)

    def tile_w2(w):  # [A, H] -> [128, KH, A]; [p, kt, a]
        wt = w.T.reshape(KH, 128, A).transpose(1, 0, 2)
        return np.ascontiguousarray(wt).astype(np.float16)

    w1t = {"agg": tile_w1(agg_W1), "adj": tile_w1(adj_W1)}
    w2t = {"agg": tile_w2(agg_W2), "adj": tile_w2(adj_W2)}
    b1 = {
        "agg": np.ascontiguousarray(agg_b1.reshape(NHT, 128).T),
        "adj": np.ascontiguousarray(adj_b1.reshape(NHT, 128).T),
    }
    b2 = {"agg": agg_b2.reshape(1, A), "adj": adj_b2.reshape(1, A)}

    mask = (np.arange(T)[None, :] < lengths[:, None]).astype(np.float32)  # [B, T]
    negmask = -mask
    negpad = (mask - 1.0) * BIG
    # stationary mask columns for the stats matmuls: [B, 16 tok-tiles, 128, BPC]
    slhs_local = np.zeros((B, 16, 128, BPC), np.float32)
    mask_t = mask.reshape(B, 16, 128)
    rank_of = np.empty(B, np.int64)
    rank_of[perm] = np.arange(B)
    for b in range(B):
        slhs_local[b, :, :, int(rank_of[b]) // NCORES] = mask_t[b]
    # fold-matmul selector: sums psum partitions {32j + b} into row b
    sel4 = np.zeros((128, BPC), np.float32)
    for j in range(NTT):
        for b in range(BPC):
            sel4[32 * j + b, b] = 1.0
    n = lengths.astype(np.float64)
    inv_n = (1.0 / n).astype(np.float32).reshape(B, 1)
    inv_nm1 = (1.0 / (n - 1.0)).astype(np.float32).reshape(B, 1)
    n_over_nm1 = (n / (n - 1.0)).astype(np.float32).reshape(B, 1)

    identb = np.eye(128, dtype=np.float16)

    in_maps = []
    for c in range(NCORES):
        sl = perm[c::NCORES]  # slot s of core c = perm[s * NCORES + c]
        im = {
            "xt": xt[sl],
            "w1t_agg": w1t["agg"], "w2t_agg": w2t["agg"],
            "w1t_adj": w1t["adj"], "w2t_adj": w2t["adj"],
            "mask": np.ascontiguousarray(
                mask[sl].reshape(BPC, 16, 128).transpose(2, 0, 1)
            ),
            "negmask": np.ascontiguousarray(
                negmask[sl].reshape(BPC, 16, 128).transpose(2, 0, 1)
            ),
            "negpad": np.ascontiguousarray(
                negpad[sl].reshape(BPC, 16, 128).transpose(2, 0, 1)
            ),
            "slhs": slhs_local[sl].transpose(2, 0, 1, 3).astype(np.float16),
            "sel4": sel4,
            "identb": identb,
            "inv_n": inv_n[sl], "inv_nm1": inv_nm1[sl],
            "n_over_nm1": n_over_nm1[sl],
        }
        if not b1_zero:
            im["b1_agg"] = b1["agg"]
            im["b1_adj"] = b1["adj"]
        if with_b2:
            im["b2_agg"] = b2["agg"]
            im["b2_adj"] = b2["adj"]
        in_maps.append(im)

    res = run_bass_kernel_spmd(nc, in_maps, core_ids=list(range(NCORES)))
    out = np.empty((B, 4 + T, A), np.float32)
    for c in range(NCORES):
        out[perm[c::NCORES]] = res.results[c]["out"]
    return out

